# revision 1
# baseline (speedup 1.0000x reference)
"""Multi-head causal attention with RoPE on 8 trn2 NeuronCores.

Problem (hardcoded): B=2, S=2048, D=2048, H=16, Hd=128, fp32.
  q/k/v = x @ wq/wk/wv; RoPE(q,k); causal softmax(q k^T/sqrt(Hd)) @ v; out @ wo.

Sharding: core c = 4*b + g handles batch b, heads [4g, 4g+4).
  - wq/wk/wv column-parallel (512 cols per core); wo column-parallel fed by an
    AllGather of per-core attention outputs o^T inside each batch group of 4
    (replaces the row-parallel all-reduce: 4MB gather instead of 16MB reduce).
  - Host-side prep: per-head even/odd column permutation of wq/wk makes RoPE
    partition-aligned in the transposed [head_dim, S] layout.
  - Causal structure: phase 1 (projection of s-block j) and phase 2
    (attention for q-block j, which only needs k/v blocks <= j) are emitted
    interleaved so the Tile scheduler can fill PE stalls across phases.
  - o^T is AllGathered in two S/2 halves; the first gather overlaps the
    second half of attention, the second overlaps the first half of the
    output projection.

All matmuls run as float32r (1 cycle/row on trn2 for moving dim >= 256).
"""
import math
import numpy as np

import concourse.bass as bass
import concourse.tile as tile
from concourse import bacc, mybir
from concourse.bass_utils import run_bass_kernel_spmd

F32 = mybir.dt.float32
F32R = mybir.dt.float32r
EXPF = mybir.ActivationFunctionType.Exp
ADD = mybir.AluOpType.add
MULT = mybir.AluOpType.mult

B, S, D = 2, 2048, 2048
H, HD = 16, 128
HPC = 4              # heads per core
DC = HPC * HD        # 512 d_out per core
NCHUNK = D // 128    # 16 contraction chunks
SB = 512             # s-block (projection and q-block granularity)
NSB = S // SB        # 4
SCALE = 1.0 / math.sqrt(HD)
NEG = -1.0e30

RG = [[0, 1, 2, 3], [4, 5, 6, 7]]


def build_module(trace_sim=False, phases=(1, 2, 3), repeat=1):
    nc = bacc.Bacc("TRN2", target_bir_lowering=False, debug=False, num_devices=8)

    x = nc.dram_tensor("x", [S, D], F32R, kind="ExternalInput").ap()
    wq = nc.dram_tensor("wq", [D, DC], F32R, kind="ExternalInput").ap()
    wk = nc.dram_tensor("wk", [D, DC], F32R, kind="ExternalInput").ap()
    wv = nc.dram_tensor("wv", [D, DC], F32R, kind="ExternalInput").ap()
    wo = nc.dram_tensor("wo", [D, DC], F32R, kind="ExternalInput").ap()
    c2 = nc.dram_tensor("c2", [128, S], F32, kind="ExternalInput").ap()
    s2n = nc.dram_tensor("s2n", [128, S], F32, kind="ExternalInput").ap()
    tri = nc.dram_tensor("tri", [128, 128], F32, kind="ExternalInput").ap()
    ones = nc.dram_tensor("ones", [128, 128], F32R, kind="ExternalInput").ap()
    ident = nc.dram_tensor("ident", [128, 128], F32R, kind="ExternalInput").ap()
    y = nc.dram_tensor("y", [S, DC], F32, kind="ExternalOutput").ap()

    ot_loc = [nc.dram_tensor(f"ot_loc{i}", [DC, S // 2], F32R) for i in range(2)]
    ot_full = [nc.dram_tensor(f"ot_full{i}", [D, S // 2], F32R) for i in range(2)]

    with tile.TileContext(nc, trace_sim=trace_sim) as tc:
        with tc.tile_pool(name="consts", bufs=1) as cpool:
            ones_t = cpool.tile([128, 128], F32R)
            nc.sync.dma_start(ones_t[:], ones[:])
            tri_t = cpool.tile([128, 128], F32)
            nc.sync.dma_start(tri_t[:], tri[:])
            id_t = cpool.tile([128, 128], F32R)
            nc.sync.dma_start(id_t[:], ident[:])
            c2_t = cpool.tile([128, S], F32)
            nc.sync.dma_start(c2_t[:], c2[:])
            s2n_t = cpool.tile([128, S], F32)
            nc.sync.dma_start(s2n_t[:], s2n[:])
            cst = dict(ones_t=ones_t, tri_t=tri_t, id_t=id_t,
                       c2_t=c2_t, s2n_t=s2n_t)

            for rep in range(repeat):
                with tc.tile_pool(name=f"qkres{rep}", bufs=1) as qkpool, \
                     tc.tile_pool(name=f"vres{rep}", bufs=1) as vpool, \
                     tc.tile_pool(name=f"p1sb{rep}", bufs=2) as p1, \
                     tc.tile_pool(name=f"p1xt{rep}", bufs=1) as p1x, \
                     tc.tile_pool(name=f"p2sb{rep}", bufs=3) as p2, \
                     tc.tile_pool(name=f"ps{rep}", bufs=1, space="PSUM") as ps:
                    qt_res = [qkpool.tile([128, S], F32R, name=f"qt{h}")
                              for h in range(HPC)]
                    kt_res = [qkpool.tile([128, S], F32R, name=f"kt{h}")
                              for h in range(HPC)]
                    v_t = [vpool.tile([128, DC], F32R, name=f"v{kb}")
                           for kb in range(S // 128)]

                    for j in range(NSB):
                        _p1_block(nc, j, x, wq, wk, wv, v_t, qt_res, kt_res,
                                  p1, p1x, ps, cst)
                    if 2 in phases:
                        for j in range(NSB):
                            _p2_block(nc, j, v_t, ot_loc, qt_res, kt_res,
                                      p2, ps, cst)
                            if 3 in phases and j == 1:
                                nc.gpsimd.collective_compute(
                                    "AllGather", mybir.AluOpType.bypass,
                                    replica_groups=RG,
                                    ins=[ot_loc[0][:]], outs=[ot_full[0][:]])
                    if 2 not in phases:
                        for h in range(HPC):
                            nc.sync.dma_start(
                                ot_loc[0][h * 128:(h + 1) * 128, :],
                                qt_res[h][:, 0:S // 2])
                            nc.sync.dma_start(
                                ot_loc[1][h * 128:(h + 1) * 128, :],
                                kt_res[h][:, 0:S // 2])
                            nc.sync.dma_start(
                                ot_loc[1][h * 128:(h + 1) * 128, 0:DC],
                                v_t[h][:])

                if 2 in phases and 3 in phases:
                    nc.gpsimd.collective_compute(
                        "AllGather", mybir.AluOpType.bypass,
                        replica_groups=RG,
                        ins=[ot_loc[1][:]], outs=[ot_full[1][:]])
                    _phase3(nc, tc, rep, ot_full, wo, y)
                else:
                    nc.sync.dma_start(y[0:DC, :], ot_loc[0][:, 0:DC].bitcast(F32))

    nc.compile()
    return nc


def _p1_block(nc, j, x, wq, wk, wv, v_t, qt_res, kt_res, p1, p1x, ps, cst):
    """Projection + RoPE for s-block j: qt/kt slices [j*SB,(j+1)*SB), v blocks."""
    s0 = j * SB
    # transpose x[s0:s0+SB, :] -> xT, split in two half-tiles (chunks 0-7 /
    # 8-15) so block j+1's transposes overlap block j's v-pass tail.
    NH = NCHUNK // 2
    xt2 = [p1x.tile([128, NH * SB], F32R, tag=f"xt{half}",
                    name=f"xt_{j}_{half}") for half in range(2)]

    def xt(c):
        return xt2[c // NH][:, (c % NH) * SB:(c % NH + 1) * SB]

    for ss in range(SB // 128):
        xrow = p1.tile([128, D], F32R, tag="xrow", bufs=2)
        # split the 1MB row-load in halves on two queues: the first 8
        # chunks' transposes start after only half the load
        nc.gpsimd.dma_start(
            xrow[:, 0:D // 2], x[s0 + ss * 128:s0 + (ss + 1) * 128, 0:D // 2])
        nc.scalar.dma_start(
            xrow[:, D // 2:], x[s0 + ss * 128:s0 + (ss + 1) * 128, D // 2:])
        for c4 in range(NCHUNK // 4):
            tp = ps.tile([128, 512], F32R, tag=f"accwk{c4 % 2}", bufs=1,
                         name=f"tp{j}_{ss}_{c4}")
            for cc in range(4):
                c = c4 * 4 + cc
                nc.tensor.transpose(
                    tp[:, cc * 128:(cc + 1) * 128],
                    xrow[:, c * 128:(c + 1) * 128], cst["id_t"][:])
            half = c4 // 2
            dst = xt2[half][:].rearrange("p (c f) -> p c f", c=NH)[
                :, (c4 % 2) * 4:(c4 % 2) * 4 + 4, ss * 128:ss * 128 + 128]
            src = tp[:].rearrange("p (c f) -> p c f", c=4)
            nc.scalar.copy(dst, src)

    # q-pass then k-pass: chunk-outer weight streaming, 4 held accumulators
    for (wsrc, res_list, wtag) in ((wq, qt_res, "wq"), (wk, kt_res, "wk")):
        prj = [ps.tile([128, SB], F32, tag=f"acc{wtag}{h}", bufs=1,
                       name=f"prj{wtag}{j}_{h}") for h in range(HPC)]
        for c in range(NCHUNK):
            wt = p1.tile([128, DC], F32R, tag=wtag, bufs=3,
                         name=f"{wtag}t{j}_{c}")
            nc.sync.dma_start(wt[:], wsrc[c * 128:(c + 1) * 128, :])
            for h in range(HPC):
                nc.tensor.matmul(
                    prj[h][:], wt[:, h * 128:(h + 1) * 128],
                    xt(c),
                    start=(c == 0), stop=(c == NCHUNK - 1))
        for h in range(HPC):
            raw = p1.tile([128, SB], F32, tag="rraw", bufs=1)
            nc.scalar.copy(raw[:], prj[h][:])
            swp = p1.tile([128, SB], F32, tag="rswp", bufs=1)
            nc.gpsimd.dma_start(swp[0:64, :], raw[64:128, :])
            nc.gpsimd.dma_start(swp[64:128, :], raw[0:64, :])
            t1 = p1.tile([128, SB], F32, tag="rt1", bufs=1)
            nc.vector.tensor_tensor(
                t1[:], raw[:], cst["c2_t"][:, s0:s0 + SB], op=MULT)
            t2 = p1.tile([128, SB], F32, tag="rt2", bufs=1)
            nc.vector.tensor_tensor(
                t2[:], swp[:], cst["s2n_t"][:, s0:s0 + SB], op=MULT)
            nc.vector.tensor_tensor(
                res_list[h][:, s0:s0 + SB], t1[:], t2[:], op=ADD)

    # v-pass (natural layout), chunk-outer, into resident v_t
    vps = [ps.tile([128, DC], F32, tag=f"accwq{ss}", bufs=1,
                   name=f"vps{j}_{ss}") for ss in range(SB // 128)]
    for c in range(NCHUNK):
        wt = p1.tile([128, DC], F32R, tag="wv", bufs=3, name=f"wvt{j}_{c}")
        nc.sync.dma_start(wt[:], wv[c * 128:(c + 1) * 128, :])
        for ss in range(SB // 128):
            nc.tensor.matmul(
                vps[ss][:], xt(c)[:, ss * 128:(ss + 1) * 128],
                wt[:], start=(c == 0), stop=(c == NCHUNK - 1))
    for ss in range(SB // 128):
        nc.scalar.copy(v_t[j * 4 + ss][:], vps[ss][:])


def _p2_block(nc, j, v_t, ot_loc, qt_res, kt_res, p2, ps, cst):
    """Causal attention for q-block j (keys/values blocks 0..4j+3)."""
    q0 = j * SB
    nkb = 4 * (j + 1)
    half = j // 2
    hq0 = q0 - half * (S // 2)
    for hp in range(HPC // 2):
        heads = (2 * hp, 2 * hp + 1)
        pv = {h: ps.tile([128, SB], F32, tag=f"accwq{h % 2}",
                         name=f"pv{j}_{h}") for h in heads}
        dn = {h: ps.tile([128, SB], F32, tag=f"accwq{2 + h % 2}",
                         name=f"dn{j}_{h}") for h in heads}
        kb_order = list(range(nkb))
        for ki, kb in enumerate(kb_order):
            r = kb - 4 * j
            if r < 0:
                lo = 0
            elif r <= 2:
                lo = r * 128
            else:
                lo = 256
            for h in heads:
                sc = ps.tile([128, SB], F32, tag=f"accwk{(kb * 2 + h) % 4}",
                             name=f"sc{j}_{h}_{kb}")
                nc.tensor.matmul(
                    sc[:, lo:], kt_res[h][:, kb * 128:(kb + 1) * 128],
                    qt_res[h][:, q0 + lo:q0 + SB],
                    start=True, stop=True)
                if r >= 0:
                    nc.vector.tensor_tensor(
                        sc[:, r * 128:(r + 1) * 128],
                        sc[:, r * 128:(r + 1) * 128], cst["tri_t"][:], op=ADD)
                if r == 3:
                    # widened dead zone: force exp() to 0 there
                    nc.vector.tensor_scalar_add(
                        sc[:, 256:384], sc[:, 256:384], NEG)
                ep = p2.tile([128, SB], F32R, tag="ep", bufs=4)
                nc.scalar.activation(ep[:, lo:], sc[:, lo:], EXPF, scale=SCALE)
                nc.tensor.matmul(
                    dn[h][:, lo:], cst["ones_t"][:], ep[:, lo:],
                    start=(kb == 0), stop=(kb == nkb - 1),
                    skip_group_check=True)
                nc.tensor.matmul(
                    pv[h][:, lo:], v_t[kb][:, h * 128:(h + 1) * 128],
                    ep[:, lo:],
                    start=(kb == 0), stop=(kb == nkb - 1),
                    skip_group_check=True)
        for h in heads:
            rec = p2.tile([128, SB], F32, tag="rec", bufs=2)
            nc.vector.reciprocal(rec[:], dn[h][:])
            ot = p2.tile([128, SB], F32R, tag="ot", bufs=2)
            nc.vector.tensor_tensor(ot[:], pv[h][:], rec[:], op=MULT)
            nc.sync.dma_start(
                ot_loc[half][h * 128:(h + 1) * 128, hq0:hq0 + SB], ot[:])


def _phase3(nc, tc, rep, ot_full, wo, y):
    """Column-parallel output projection from gathered o^T halves."""
    with tc.tile_pool(name=f"p3wo{rep}", bufs=1) as p3w, \
         tc.tile_pool(name=f"p3sb{rep}", bufs=3) as p3, \
         tc.tile_pool(name=f"p3ps{rep}", bufs=2, space="PSUM") as p3ps:
        wo_t = [p3w.tile([128, DC], F32R, name=f"wo{c}") for c in range(NCHUNK)]
        for c in range(NCHUNK):
            nc.sync.dma_start(wo_t[c][:], wo[c * 128:(c + 1) * 128, :])
        for sq in range(4):
            o0 = sq * 512
            half = sq // 2
            ho0 = o0 - half * (S // 2)
            otf = [p3.tile([128, 512], F32R, tag=f"otf{c}", bufs=2,
                           name=f"otf{c}_{sq}") for c in range(NCHUNK)]
            for c in range(NCHUNK):
                nc.sync.dma_start(
                    otf[c][:],
                    ot_full[half][c * 128:(c + 1) * 128, ho0:ho0 + 512])
            for ss in range(4):
                yps = p3ps.tile([128, DC], F32, tag="yps")
                for c in range(NCHUNK):
                    nc.tensor.matmul(
                        yps[:], otf[c][:, ss * 128:(ss + 1) * 128],
                        wo_t[c][:], start=(c == 0), stop=(c == NCHUNK - 1))
                ysb = p3.tile([128, DC], F32, tag="ysb")
                nc.scalar.copy(ysb[:], yps[:])
                nc.sync.dma_start(
                    y[o0 + ss * 128:o0 + (ss + 1) * 128, :], ysb[:])


_PERM = np.concatenate([np.arange(0, 128, 2), np.arange(1, 128, 2)])


def make_in_maps(x, wq, wk, wv, wo, freqs_cos, freqs_sin):
    """Host-side sharding/prep. Returns list of 8 per-core input dicts."""
    cosT = np.ascontiguousarray(freqs_cos.T.astype(np.float32))   # [64, S]
    sinT = np.ascontiguousarray(freqs_sin.T.astype(np.float32))
    c2 = np.concatenate([cosT, cosT], axis=0)                     # [128, S]
    s2n = np.concatenate([-sinT, sinT], axis=0)
    tri = np.where(np.arange(128)[None, :] >= np.arange(128)[:, None],
                   0.0, NEG).astype(np.float32)                   # [k, q]
    ones = np.ones((128, 128), dtype=np.float32)
    ident = np.eye(128, dtype=np.float32)

    in_maps = []
    for c in range(8):
        b, g = divmod(c, 4)
        cols = slice(g * DC, (g + 1) * DC)
        wq_c = np.ascontiguousarray(wq[:, cols]).copy()
        wk_c = np.ascontiguousarray(wk[:, cols]).copy()
        for h in range(HPC):
            blk = slice(h * 128, (h + 1) * 128)
            wq_c[:, blk] = wq_c[:, blk][:, _PERM]
            wk_c[:, blk] = wk_c[:, blk][:, _PERM]
        in_maps.append({
            "x": np.ascontiguousarray(x[b]).astype(np.float32),
            "wq": wq_c.astype(np.float32),
            "wk": wk_c.astype(np.float32),
            "wv": np.ascontiguousarray(wv[:, cols]).astype(np.float32),
            "wo": np.ascontiguousarray(wo[:, cols]).astype(np.float32),
            "c2": c2, "s2n": s2n, "tri": tri, "ones": ones, "ident": ident,
        })
    return in_maps


def assemble(results):
    """Concatenate per-core column outputs into [B, S, D]."""
    out = np.empty((B, S, D), dtype=np.float32)
    for c in range(8):
        b, g = divmod(c, 4)
        out[b][:, g * DC:(g + 1) * DC] = results[c]["y"]
    return out


_NC = None


def kernel(x, wq, wk, wv, wo, freqs_cos, freqs_sin):
    global _NC
    x = np.asarray(x); wq = np.asarray(wq); wk = np.asarray(wk)
    wv = np.asarray(wv); wo = np.asarray(wo)
    freqs_cos = np.asarray(freqs_cos); freqs_sin = np.asarray(freqs_sin)
    if _NC is None:
        _NC = build_module()
    in_maps = make_in_maps(x, wq, wk, wv, wo, freqs_cos, freqs_sin)
    res = run_bass_kernel_spmd(_NC, in_maps, core_ids=list(range(8)))
    return assemble(res.results)

